# revision 25
# baseline (speedup 1.0000x reference)
"""Trainium2 Bass kernel for nn_NeuralODESolver (Tsit5 neural-ODE integrator).

Strategy (data-parallel across 8 NeuronCores, 2-way interleaved per core):
  - Shard the batch dim (1024) into 8 x 128; each core splits its batch into
    two fully independent 64-wide halves (A/B) whose Tsit5 chains interleave
    with a one-time startup skew, so the tensor engine always has ready work
    and the HAM clock gate holds the warm 2.4 GHz state. No tile is shared
    between the halves -- shared tiles re-couple the chains into lockstep.
  - Plain Tsit5 stage structure per half: L1 (2 MM) -> relu1 -> L2 (4 MM) ->
    relu2 -> L3 dup [k;k] (2 MM) -> one fp16 z-write + dup-paired fp32
    accumulator updates on the vector engine.
  - Bias/forcing folding: z tiles are [y; I64] (or [I64; y]) in fp16; the L1
    stationary is [W1y^T ; (W1u@u + b1)^T] so the constant forcing and b1
    enter through the identity carrier -- u never ships to the device and
    relu1 needs no bias. b2 prefills PSUM via one identity matmul per stage
    (skipped when b2 == 0), so both relus are single zero-bias merged
    [128, 2x64] ACT ops.
  - PSUM has_written is cleared bank-wide by any start=True matmul, so each
    bank gets exactly one start=True (first MM), later regions write onto
    cleared elements with start=False, explicitly ordered after the clear.
"""

import numpy as np

# Tsitouras 5(4) tableau (5th-order weights; b7 = 0)
_A21 = 0.161
_A31, _A32 = -0.008480655492356989, 0.335480655492357
_A41, _A42, _A43 = 2.8971530571054935, -6.359448489975075, 4.3622954328695815
_A51, _A52, _A53, _A54 = 5.325864828439257, -11.748883564062828, 7.4955393428898365, -0.09249506636175525
_A61, _A62, _A63, _A64, _A65 = 5.86145544294642, -12.92096931784711, 8.159367898576159, -0.071584973281401, -0.028269050394068383
_B1, _B2, _B3, _B4, _B5, _B6 = 0.09646076681806523, 0.01, 0.4798896504144996, 1.379008574103742, -3.290069515436081, 2.324710524099774

SECOND = 1.0 / 3600.0
DT0 = 60.0

N_CORES = 8
NH = 64  # half-batch width per core

N_WARMUP_MM = 18

_ZVAR = {1: 0, 2: 0, 3: 0, 4: 1, 5: 0, 6: 1}  # 0 = [y;I], 1 = [I;y]


def _build_program(n_steps, b2_nonzero, b3_nonzero):
    import concourse.mybir as mybir
    import concourse.tile as tile
    from concourse.tile import add_dep_helper
    from concourse import bacc

    f16 = mybir.dt.float16
    f32 = mybir.dt.float32
    Relu = mybir.ActivationFunctionType.Relu
    MUL = mybir.AluOpType.mult
    ADD = mybir.AluOpType.add

    h = DT0 * SECOND
    hA = {
        (2, 1): h * _A21,
        (3, 1): h * _A31, (3, 2): h * _A32,
        (4, 1): h * _A41, (4, 2): h * _A42, (4, 3): h * _A43,
        (5, 1): h * _A51, (5, 2): h * _A52, (5, 3): h * _A53, (5, 4): h * _A54,
        (6, 1): h * _A61, (6, 2): h * _A62, (6, 3): h * _A63, (6, 4): h * _A64, (6, 5): h * _A65,
    }
    hB = {j: h * v for j, v in enumerate((_B1, _B2, _B3, _B4, _B5, _B6), start=1)}

    nc = bacc.Bacc()

    y0_d = nc.declare_dram_parameter("y0", [64, 2 * NH], f32, isOutput=False)
    y016_d = nc.declare_dram_parameter("y016", [64, 2 * NH], f16, isOutput=False)
    id64_d = nc.declare_dram_parameter("id64", [64, 64], f16, isOutput=False)
    w1cu_d = {
        (v, m, x): nc.declare_dram_parameter(f"w1cu{v}{m}{x}", [128, 128], f16, isOutput=False)
        for v in range(2) for m in range(2) for x in range(2)
    }
    w2t_d = nc.declare_dram_parameter("w2t", [128, 512], f16, isOutput=False)
    w3td_d = nc.declare_dram_parameter("w3td", [128, 256], f16, isOutput=False)
    cv_d = nc.declare_dram_parameter("cv", [128, 4], f32, isOutput=False)
    if b2_nonzero:
        cb2_d = nc.declare_dram_parameter("cb2", [128, 128], f16, isOutput=False)
    if b3_nonzero:
        cb3_d = nc.declare_dram_parameter("cb3", [128, 64], f16, isOutput=False)
    yout_d = nc.declare_dram_parameter("yout", [64, 2 * NH], f32, isOutput=True)

    with tile.TileContext(nc) as tc:
        with (
            tc.tile_pool(name="const", bufs=1) as cpool,
            tc.tile_pool(name="state", bufs=1) as spool,
            tc.tile_pool(name="act", bufs=2) as apool,
            tc.tile_pool(name="psum", bufs=2, space="PSUM") as ppool,
        ):
            # ---- constants ----
            w1cu = {k: cpool.tile([128, 128], f16, name=f"w1cu{k[0]}{k[1]}{k[2]}") for k in w1cu_d}
            w2t = cpool.tile([128, 512], f16)
            w3td = cpool.tile([128, 256], f16)
            cv = cpool.tile([128, 4], f32)
            zerot = cpool.tile([128, 128], f16)
            need_ident = b2_nonzero or b3_nonzero
            if need_ident:
                ident = cpool.tile([128, 128], f16)
                nc.sync.dma_start(ident[0:64, 0:64], id64_d[:])
                nc.sync.dma_start(ident[64:128, 64:128], id64_d[:])
                nc.gpsimd.memset(ident[0:64, 64:128], 0.0)
                nc.gpsimd.memset(ident[64:128, 0:64], 0.0)
            if b2_nonzero:
                cb2 = cpool.tile([128, 128], f16)
                nc.sync.dma_start(cb2[:], cb2_d[:])
            if b3_nonzero:
                cb3 = cpool.tile([128, 64], f16)
                nc.sync.dma_start(cb3[:], cb3_d[:])

            # warmup matmuls while the DMAs land (PSUM banks all spoken for,
            # so reuse the pa1 tags)
            nc.gpsimd.memset(zerot[:], 0.0)
            for i in range(N_WARMUP_MM):
                pwarm = ppool.tile([128, 128], f32, tag=f"pa1_{i % 2}", bufs=1, name="pwarm")
                nc.tensor.matmul(pwarm[:], zerot[:], zerot[:], start=True, stop=True)

            for k in w1cu:
                nc.sync.dma_start(w1cu[k][:], w1cu_d[k][:])
            nc.sync.dma_start(w2t[:], w2t_d[:])
            nc.sync.dma_start(w3td[:], w3td_d[:])
            nc.sync.dma_start(cv[:], cv_d[:])

            cv34_1 = cv[:, 0:1]
            cv56_1 = cv[:, 1:2]
            cv56_2 = cv[:, 2:3]
            cv56_3 = cv[:, 3:4]

            # ---- per-half state (NOTHING shared between halves) ----
            state = []
            for x in range(2):  # halves A=0 (batch cols 0:64), B=1 (64:128)
                st = {}
                st["ycur"] = spool.tile([128, NH], f32, name=f"ycur{x}")
                st["ynew"] = spool.tile([128, NH], f32, name=f"ynew{x}")
                st["P34"] = spool.tile([128, NH], f32, name=f"P34_{x}")
                st["P56"] = spool.tile([128, NH], f32, name=f"P56_{x}")
                st["z"] = {j: spool.tile([128, NH], f16, name=f"z{j}_{x}") for j in range(1, 7)}
                cols = slice(x * NH, (x + 1) * NH)
                nc.sync.dma_start(st["ycur"][0:64, :], y0_d[:, cols])
                nc.sync.dma_start(st["ycur"][64:128, :], y0_d[:, cols])
                nc.sync.dma_start(st["z"][1][0:64, :], y016_d[:, cols])
                for j in range(1, 7):
                    if _ZVAR[j] == 0:
                        nc.sync.dma_start(st["z"][j][64:128, :], id64_d[:])
                    else:
                        nc.sync.dma_start(st["z"][j][0:64, :], id64_d[:])
                state.append(st)

            def stt_v(out, in0, scal, in1):
                nc.vector.scalar_tensor_tensor(out, in0, scal, in1, op0=MUL, op1=ADD)

            LO = slice(0, 64)
            HI = slice(64, 128)
            skew = {"a_relu2": None, "done": False}

            def emit_stage(x, j, step, last_step):
                st = state[x]
                ycur, ynew, P34, P56 = st["ycur"], st["ynew"], st["P34"], st["P56"]

                # L1: pre1 = [W1y ; cu1] @ [y; I]  (both m-blocks, one tile).
                # start=True clears has_written bank-wide: only the first MM
                # carries it; the m1 block writes onto cleared elements and is
                # explicitly ordered after the clearing MM.
                v = _ZVAR[j]
                zt = st["z"][j]
                pa1 = ppool.tile([128, 2 * NH], f32, tag=f"pa1_{x}", bufs=1, name=f"pa1_{x}")
                mm_l1 = nc.tensor.matmul(pa1[:, 0:NH], w1cu[(v, 0, x)][:], zt[:], start=True, stop=True)
                if x == 1 and not skew["done"] and skew["a_relu2"] is not None:
                    # one-time startup skew: hold half B ~half a stage behind
                    # half A so the chains dovetail instead of locking in phase
                    add_dep_helper(mm_l1.ins, skew["a_relu2"].ins, sync=True, reason="AB skew")
                    skew["done"] = True
                mm_l1b = nc.tensor.matmul(pa1[:, NH:2 * NH], w1cu[(v, 1, x)][:], zt[:], start=False, stop=True,
                                          skip_group_check=True)
                add_dep_helper(mm_l1b.ins, mm_l1.ins, sync=False, reason="bank clear order")

                a1 = apool.tile([128, 2 * NH], f16, tag=f"a1_{x}", name=f"a1_{x}")
                nc.scalar.activation(a1[:], pa1[:], Relu)

                # L2: pre2 = W2 @ a1 (K=256 over the two a1 col-blocks)
                pa2 = ppool.tile([128, 2 * NH], f32, tag=f"pa2_{x}", bufs=1, name=f"pa2_{x}")
                if b2_nonzero:
                    mm_c = nc.tensor.matmul(pa2[:], ident[:], cb2[:], start=True, stop=False)
                    st2 = False
                else:
                    st2 = True
                mm_k0m0 = nc.tensor.matmul(pa2[:, 0:NH], w2t[:, 0:128], a1[:, 0:NH], start=st2, stop=False,
                                           skip_group_check=True)
                if b2_nonzero:
                    add_dep_helper(mm_k0m0.ins, mm_c.ins, sync=False, reason="bank clear order")
                    first = mm_c
                else:
                    first = mm_k0m0
                mm_k0m1 = nc.tensor.matmul(pa2[:, NH:2 * NH], w2t[:, 128:256], a1[:, 0:NH], start=False, stop=False,
                                           skip_group_check=True)
                add_dep_helper(mm_k0m1.ins, first.ins, sync=False, reason="bank clear order")
                nc.tensor.matmul(pa2[:, 0:NH], w2t[:, 256:384], a1[:, NH:2 * NH], start=False, stop=True,
                                 skip_group_check=True)
                nc.tensor.matmul(pa2[:, NH:2 * NH], w2t[:, 384:512], a1[:, NH:2 * NH], start=False, stop=True,
                                 skip_group_check=True)

                a2 = apool.tile([128, 2 * NH], f16, tag=f"a2_{x}", name=f"a2_{x}")
                r2 = nc.scalar.activation(a2[:], pa2[:], Relu)
                if x == 0 and j == 1 and step == 0:
                    skew["a_relu2"] = r2

                # L3: pk = [k; k] = [W3|W3] @ a2 (own bank per half)
                pk = ppool.tile([128, NH], f32, tag=f"pk_{x}", bufs=2, name=f"pk_{x}")
                if b3_nonzero:
                    mm_b3 = nc.tensor.matmul(pk[:], ident[:], cb3[:], start=True, stop=False)
                    mm_l3a = nc.tensor.matmul(pk[:], w3td[:, 0:128], a2[:, 0:NH], start=False, stop=False,
                                              skip_group_check=True)
                    add_dep_helper(mm_l3a.ins, mm_b3.ins, sync=False, reason="bank clear order")
                else:
                    nc.tensor.matmul(pk[:], w3td[:, 0:128], a2[:, 0:NH], start=True, stop=False)
                nc.tensor.matmul(pk[:], w3td[:, 128:256], a2[:, NH:2 * NH], start=False, stop=True)

                # on-chain z-write (DVE, PSUM source)
                if j == 1:
                    stt_v(st["z"][2][LO, :], pk[LO, :], hA[(2, 1)], ycur[LO, :])
                elif j == 2:
                    stt_v(st["z"][3][LO, :], pk[LO, :], hA[(3, 2)], P34[LO, :])
                elif j == 3:
                    stt_v(st["z"][4][HI, :], pk[HI, :], hA[(4, 3)], P34[HI, :])
                elif j == 4:
                    stt_v(st["z"][5][LO, :], pk[LO, :], hA[(5, 4)], P56[LO, :])
                elif j == 5:
                    stt_v(st["z"][6][HI, :], pk[HI, :], hA[(6, 5)], P56[HI, :])
                else:  # j == 6
                    if not last_step:
                        stt_v(st["z"][1][LO, :], pk[LO, :], hB[6], ynew[LO, :])

                # off-chain fp32 accumulator updates (DVE, dup-paired)
                if j == 1:
                    stt_v(P34[:], pk[:], cv34_1, ycur[:])
                    stt_v(P56[:], pk[:], cv56_1, ycur[:])
                    stt_v(ynew[:], pk[:], hB[1], ycur[:])
                elif j == 2:
                    stt_v(P34[HI, :], pk[HI, :], hA[(4, 2)], P34[HI, :])
                    stt_v(P56[:], pk[:], cv56_2, P56[:])
                    stt_v(ynew[:], pk[:], hB[2], ynew[:])
                elif j == 3:
                    stt_v(P56[:], pk[:], cv56_3, P56[:])
                    stt_v(ynew[:], pk[:], hB[3], ynew[:])
                elif j == 4:
                    stt_v(P56[HI, :], pk[HI, :], hA[(6, 4)], P56[HI, :])
                    stt_v(ynew[:], pk[:], hB[4], ynew[:])
                elif j == 5:
                    stt_v(ynew[:], pk[:], hB[5], ynew[:])
                else:  # j == 6: ynew becomes y for the next step
                    stt_v(ynew[:], pk[:], hB[6], ynew[:])

            for step in range(n_steps):
                last_step = step == n_steps - 1
                for j in range(1, 7):
                    emit_stage(0, j, step, last_step)
                    emit_stage(1, j, step, last_step)
                for x in range(2):
                    st = state[x]
                    st["ycur"], st["ynew"] = st["ynew"], st["ycur"]

            for x in range(2):
                cols = slice(x * NH, (x + 1) * NH)
                nc.sync.dma_start(yout_d[:, cols], state[x]["ycur"][0:64, :])

    nc.compile()
    return nc


def kernel(x0, u, W1, b1, W2, b2, W3, b3, t0, t1):
    from concourse.bass_utils import run_bass_kernel_spmd

    x0 = np.asarray(x0, dtype=np.float32)
    u = np.asarray(u, dtype=np.float32)
    W1 = np.asarray(W1, dtype=np.float32)
    W2 = np.asarray(W2, dtype=np.float32)
    W3 = np.asarray(W3, dtype=np.float32)
    b1 = np.asarray(b1, dtype=np.float32)
    b2 = np.asarray(b2, dtype=np.float32)
    b3 = np.asarray(b3, dtype=np.float32)

    Bt, D = x0.shape
    n = Bt // N_CORES
    h = DT0 * SECOND
    n_steps = int(round((float(np.asarray(t1)) - float(np.asarray(t0))) / h))
    b2_nonzero = bool(np.any(b2 != 0))
    b3_nonzero = bool(np.any(b3 != 0))

    nc = _build_program(n_steps, b2_nonzero, b3_nonzero)

    f16 = np.float16
    W1y = W1[:, 0:64]
    W1u = W1[:, 64:128]

    w2T = W2.T.astype(f16)
    w2t = np.ascontiguousarray(
        np.concatenate([w2T[0:128, 0:128], w2T[0:128, 128:256], w2T[128:256, 0:128], w2T[128:256, 128:256]], axis=1)
    )
    w3T = W3.T.astype(f16)
    w3td = np.ascontiguousarray(
        np.concatenate([w3T[0:128], w3T[0:128], w3T[128:256], w3T[128:256]], axis=1)
    )

    cvm = np.zeros((128, 4), np.float32)
    cvm[0:64, 0] = h * _A31
    cvm[64:128, 0] = h * _A41
    cvm[0:64, 1] = h * _A51
    cvm[64:128, 1] = h * _A61
    cvm[0:64, 2] = h * _A52
    cvm[64:128, 2] = h * _A62
    cvm[0:64, 3] = h * _A53
    cvm[64:128, 3] = h * _A63

    id64 = np.eye(64, dtype=f16)

    in_maps = []
    for c in range(N_CORES):
        sl = slice(c * n, (c + 1) * n)
        x0c = x0[sl]
        uc = u[sl]
        im = {
            "y0": np.ascontiguousarray(x0c.T),
            "y016": np.ascontiguousarray(x0c.T.astype(f16)),
            "id64": id64,
            "w2t": w2t,
            "w3td": w3td,
            "cv": cvm,
        }
        if b2_nonzero:
            cb2 = np.zeros((128, 128), np.float32)
            cb2[:, 0:64] = b2[0:128, None]
            cb2[:, 64:128] = b2[128:256, None]
            im["cb2"] = cb2.astype(f16)
        if b3_nonzero:
            cb3 = np.zeros((128, 64), np.float32)
            cb3[0:64] = b3[:, None]
            cb3[64:128] = b3[:, None]
            im["cb3"] = cb3.astype(f16)
        for x in range(2):
            ux = uc[x * NH:(x + 1) * NH]
            cu1 = W1u @ ux.T + b1[:, None]
            for m in range(2):
                w1yT = W1y.T[:, m * 128:(m + 1) * 128]
                cu1T = cu1[m * 128:(m + 1) * 128, :].T
                im[f"w1cu0{m}{x}"] = np.ascontiguousarray(
                    np.concatenate([w1yT, cu1T], axis=0).astype(f16))
                im[f"w1cu1{m}{x}"] = np.ascontiguousarray(
                    np.concatenate([cu1T, w1yT], axis=0).astype(f16))
        in_maps.append(im)

    res = run_bass_kernel_spmd(nc, in_maps, list(range(N_CORES)))
    globals()["LAST_RESULT"] = res

    out = np.empty((Bt, D), np.float32)
    for c in range(N_CORES):
        out[c * n:(c + 1) * n, :] = res.results[c]["yout"].T
    return out


# revision 26
# speedup vs baseline: 1.1549x; 1.1549x over previous
"""Trainium2 Bass kernel for nn_NeuralODESolver (Tsit5 neural-ODE integrator).

Strategy (data-parallel across 8 NeuronCores):
  - Shard the batch dim (1024) into 8 x 128; MLP weights replicated.
  - Feature-major layout on device: activations are [features(partitions), batch(free)].
  - Matmul operands in fp16 (full PE rate; validated ~2e-4 rel err vs fp32 ref);
    PSUM accumulation and all Runge-Kutta state arithmetic in fp32.
  - ReLU + bias fused into the PSUM->SBUF copy on the scalar (ACT) engine.
  - Layer 3 is algebraically fused into the NEXT stage's layer 1 via
    FW = W1y@W3 (host-precomputed, scaled by the Butcher coefficient):
    pre1_t = W1@[zbase_t; u] + cext*FW@a2_{t-1}. The base matmuls and all
    k-scatters run off the critical path; the chain is just
    relu -> L2 -> relu -> ext-matmuls.
  - L3 still computes k with a duplicated stationary operand [W3^T | W3^T]
    ([k; k] on 128 partitions) so RK scatters update two fp32 accumulator
    targets per fused scalar_tensor_tensor op; accumulator updates are
    deferred one stage so they queue behind the next stage's relus in the
    vector-engine FIFO.
  - Stage-input tiles z4/z6 use a flipped [u; y] layout (with a row-swapped
    W1^T) so every scatter op stays partition-aligned.
"""

import numpy as np

# Tsitouras 5(4) tableau (5th-order weights; b7 = 0)
_A21 = 0.161
_A31, _A32 = -0.008480655492356989, 0.335480655492357
_A41, _A42, _A43 = 2.8971530571054935, -6.359448489975075, 4.3622954328695815
_A51, _A52, _A53, _A54 = 5.325864828439257, -11.748883564062828, 7.4955393428898365, -0.09249506636175525
_A61, _A62, _A63, _A64, _A65 = 5.86145544294642, -12.92096931784711, 8.159367898576159, -0.071584973281401, -0.028269050394068383
_B1, _B2, _B3, _B4, _B5, _B6 = 0.09646076681806523, 0.01, 0.4798896504144996, 1.379008574103742, -3.290069515436081, 2.324710524099774

SECOND = 1.0 / 3600.0
DT0 = 60.0

N_CORES = 8

_A = {
    (2, 1): _A21,
    (3, 1): _A31, (3, 2): _A32,
    (4, 1): _A41, (4, 2): _A42, (4, 3): _A43,
    (5, 1): _A51, (5, 2): _A52, (5, 3): _A53, (5, 4): _A54,
    (6, 1): _A61, (6, 2): _A62, (6, 3): _A63, (6, 4): _A64, (6, 5): _A65,
}
_B = {1: _B1, 2: _B2, 3: _B3, 4: _B4, 5: _B5, 6: _B6}


def _build_program(n, n_steps, b3_nonzero):
    import concourse.bass as bass  # noqa: F401
    import concourse.mybir as mybir
    import concourse.tile as tile
    from concourse.tile import add_dep_helper
    from concourse import bacc

    f32 = mybir.dt.float32
    f16 = mybir.dt.float16
    Relu = mybir.ActivationFunctionType.Relu
    Copy = mybir.ActivationFunctionType.Copy
    MUL = mybir.AluOpType.mult
    ADD = mybir.AluOpType.add
    MAX = mybir.AluOpType.max

    h = DT0 * SECOND
    C = {k: h * v for k, v in _A.items()}
    HB = {k: h * v for k, v in _B.items()}

    # Keep data waits on the MATMUL rather than letting bacc move them onto
    # LDWEIGHTS: an unblocked LDWEIGHTS can be pulled ahead by the PE's
    # reorder window and prefetch weights during dependency stalls.
    # generate_event_semaphores still enforces the 1-wait-per-instruction
    # hardware constraint by splitting through event semaphores.
    nc = bacc.Bacc()

    y0_d = nc.declare_dram_parameter("y0", [64, n], f32, isOutput=False)
    u16_d = nc.declare_dram_parameter("u16", [64, n], f16, isOutput=False)
    w1t_d = nc.declare_dram_parameter("w1t", [128, 256], f16, isOutput=False)
    w1tf_d = nc.declare_dram_parameter("w1tf", [128, 256], f16, isOutput=False)
    w2t_d = nc.declare_dram_parameter("w2t", [128, 512], f16, isOutput=False)
    w3td_d = nc.declare_dram_parameter("w3td", [128, 256], f16, isOutput=False)
    fw_d = [nc.declare_dram_parameter(f"fw{j}", [128, 512], f16, isOutput=False) for j in range(6)]
    bb_d = nc.declare_dram_parameter("bb", [128, 19], f32, isOutput=False)
    cv_d = nc.declare_dram_parameter("cv", [128, 1], f32, isOutput=False)
    yout_d = nc.declare_dram_parameter("yout", [64, n], f32, isOutput=True)

    with tile.TileContext(nc) as tc:
        with (
            tc.tile_pool(name="const", bufs=1) as cpool,
            tc.tile_pool(name="state", bufs=1) as spool,
            tc.tile_pool(name="act", bufs=2) as apool,
            tc.tile_pool(name="psum", bufs=2, space="PSUM") as ppool,
        ):
            w1t = cpool.tile([128, 256], f16)
            w1tf = cpool.tile([128, 256], f16)
            w2t = cpool.tile([128, 512], f16)
            w3td = cpool.tile([128, 256], f16)
            fw = [cpool.tile([128, 512], f16, name=f"fw{j}") for j in range(6)]
            bb = cpool.tile([128, 19], f32)
            cv = cpool.tile([128, 1], f32)
            zerot = cpool.tile([128, n], f32)

            ydup = spool.tile([128, n], f32)
            ynewd = spool.tile([128, n], f32)
            p45 = spool.tile([128, n], f32)   # [zb5 acc (0:64); zb4 acc (64:128)]
            zb6t = spool.tile([128, n], f32)  # zb6 acc in 64:128
            z = {i: spool.tile([128, n], f16, name=f"z{i}") for i in range(1, 7)}
            # activation tiles are allocated per stage from a double-buffered
            # pool: the relu writes then carry no same-buffer WAR hazard, so
            # each needs only a single PE-semaphore wait (no event-semaphore
            # relay that would anchor it to the end of the whole matmul group)

            nc.sync.dma_start(w1t[:], w1t_d[:])
            nc.sync.dma_start(w1tf[:], w1tf_d[:])
            nc.sync.dma_start(w2t[:], w2t_d[:])
            nc.sync.dma_start(w3td[:], w3td_d[:])
            for j in range(6):
                nc.sync.dma_start(fw[j][:], fw_d[j][:])
            nc.sync.dma_start(bb[:], bb_d[:])
            nc.sync.dma_start(cv[:], cv_d[:])
            nc.gpsimd.memset(zerot[:], 0.0)

            nc.sync.dma_start(ydup[0:64, :], y0_d[:])
            nc.sync.dma_start(ydup[64:128, :], y0_d[:])
            # u halves of the stage-input tiles: z4/z6 are flipped ([u; y]).
            for i in (1, 2, 3, 5):
                nc.sync.dma_start(z[i][64:128, :], u16_d[:])
            for i in (4, 6):
                nc.sync.dma_start(z[i][0:64, :], u16_d[:])
            # y halves of z1/z2 (fp16 cast of initial state; z2base = y0 too)
            nc.scalar.activation(z[1][0:64, :], ydup[0:64, :], Copy)
            nc.scalar.activation(z[2][0:64, :], ydup[0:64, :], Copy)

            # bb columns: 0,1 plain b1 lo/hi; 2+2t,3+2t eff-b1 per stage t=1..6
            # (b1 + cext*W1y@b3); 14,15 b2 lo/hi; 16 b3
            b1plain = (bb[:, 0:1], bb[:, 1:2])
            b1eff = {t: (bb[:, 2 + 2 * t : 3 + 2 * t], bb[:, 3 + 2 * t : 4 + 2 * t]) for t in range(6)}
            b2lo, b2hi = bb[:, 14:15], bb[:, 15:16]
            b3v = bb[:, 16:17]

            # which W1 variant and where the y half lives, per stage
            flipped = {1: False, 2: False, 3: False, 4: True, 5: False, 6: True}

            def stt(out, in0, scal, in1):
                nc.vector.scalar_tensor_tensor(out, in0, scal, in1, op0=MUL, op1=ADD)

            # Stage pipeline with layer-3 fused into the next stage's
            # layer-1 via FW = W1y@W3 (host-precomputed, scaled per stage):
            #   pre1_{t} = W1 @ [zbase_t; u]  (base MMs, off critical path)
            #            + cext * FW @ a2_{t-1}  (ext MMs, on critical path)
            # zbase_t excludes the k_{t-1} term, so its fp16 write happens a
            # full stage early. k-scatters feed only zbase accumulators and
            # run off-chain: the one fp16 z-final per stage on the vector
            # engine (PSUM source), fp32 accumulator updates on GPSIMD from
            # an SBUF copy of k.
            def new_pa1():
                return (
                    ppool.tile([128, n], f32, tag="pa1m0", bufs=2, name="pa1m0"),
                    ppool.tile([128, n], f32, tag="pa1m1", bufs=2, name="pa1m1"),
                )

            # prologue: full layer-1 for step 0 stage 1 (no ext contribution)
            pa1 = new_pa1()
            nc.tensor.matmul(pa1[0][:], w1t[:, 0:128], z[1][:], start=True, stop=True)
            nc.tensor.matmul(pa1[1][:], w1t[:, 128:256], z[1][:], start=True, stop=True)
            cur_bias = b1plain

            # fp32 accumulator updates are deferred one block so they queue
            # BEHIND the next stage's relu ops in the vector-engine FIFO
            pending_accs = []

            for step in range(n_steps):
                last_step = step == n_steps - 1
                for i in range(1, 7):
                    pa1m0, pa1m1 = pa1

                    pa2m0 = ppool.tile([128, n], f32, tag="pa2m0", bufs=1)
                    pa2m1 = ppool.tile([128, n], f32, tag="pa2m1", bufs=1)
                    pk = ppool.tile([128, n], f32, tag="pk", bufs=2)

                    # relu of this stage's pre1
                    a1lo = apool.tile([128, n], f16, tag="a1lo", name="a1lo")
                    a1hi = apool.tile([128, n], f16, tag="a1hi", name="a1hi")
                    a2lo = apool.tile([128, n], f16, tag="a2lo", name="a2lo")
                    a2hi = apool.tile([128, n], f16, tag="a2hi", name="a2hi")
                    nc.scalar.activation(a1lo[:], pa1m0[:], Relu, bias=cur_bias[0])
                    nc.vector.tensor_scalar(a1hi[:], pa1m1[:], cur_bias[1], 0.0, op0=ADD, op1=MAX)

                    # flush previous stage's accumulator updates
                    for fn in pending_accs:
                        fn()
                    pending_accs = []

                    # layer 2: pre2 = W2 @ a1 (K=256 in two accumulating
                    # halves); the m0-half relu is emitted between the m0 and
                    # m1 matmul pairs so its wait anchors to the m0 close, not
                    # the whole group
                    nc.tensor.matmul(pa2m0[:], w2t[:, 0:128], a1lo[:], start=True, stop=False)
                    mm_m0k1 = nc.tensor.matmul(pa2m0[:], w2t[:, 256:384], a1hi[:], start=False, stop=True)
                    nc.scalar.activation(a2lo[:], pa2m0[:], Relu, bias=b2lo)
                    mm_m1k0 = nc.tensor.matmul(pa2m1[:], w2t[:, 128:256], a1lo[:], start=True, stop=False)
                    nc.tensor.matmul(pa2m1[:], w2t[:, 384:512], a1hi[:], start=False, stop=True)
                    nc.vector.tensor_scalar(a2hi[:], pa2m1[:], b2hi, 0.0, op0=ADD, op1=MAX)
                    # keep the m0 group closing as the SECOND matmul: without
                    # this edge the scheduler slots m1k0 (ready earlier) ahead
                    # of m0k1, pushing the m0 close -- and the a2lo relu the
                    # chain runs through -- one matmul later
                    add_dep_helper(mm_m1k0.ins, mm_m0k1.ins, sync=False, reason="close pa2m0 early")

                    # base + ext matmuls building the NEXT stage's pre1
                    if not (last_step and i == 6):
                        t = i + 1 if i < 6 else 1
                        w1v = w1tf if flipped[t] else w1t
                        zt = z[t]
                        V = fw[i - 1]
                        npa1 = new_pa1()
                        nc.tensor.matmul(npa1[0][:], w1v[:, 0:128], zt[:], start=True, stop=False)
                        nc.tensor.matmul(npa1[1][:], w1v[:, 128:256], zt[:], start=True, stop=False)
                        nc.tensor.matmul(npa1[0][:], V[:, 0:128], a2lo[:], start=False, stop=False)
                        ext_m0k1 = nc.tensor.matmul(npa1[0][:], V[:, 256:384], a2hi[:], start=False, stop=True)
                        ext_m1k0 = nc.tensor.matmul(npa1[1][:], V[:, 128:256], a2lo[:], start=False, stop=False)
                        nc.tensor.matmul(npa1[1][:], V[:, 384:512], a2hi[:], start=False, stop=True)
                        add_dep_helper(ext_m1k0.ins, ext_m0k1.ins, sync=False, reason="close pa1m0 early")
                        pa1 = npa1
                        cur_bias = b1eff[t - 1]

                    # layer 3 (duplicated): pk = [k; k] = [W3|W3] @ a2
                    if b3_nonzero:
                        nc.vector.tensor_scalar_add(pk[:], zerot[:], b3v)
                        nc.tensor.matmul(pk[:], w3td[:, 0:128], a2lo[:], start=False, stop=False)
                    else:
                        nc.tensor.matmul(pk[:], w3td[:, 0:128], a2lo[:], start=True, stop=False)
                    nc.tensor.matmul(pk[:], w3td[:, 128:256], a2hi[:], start=False, stop=True)

                    # one fp16 zbase final write per stage now (reads PSUM);
                    # fp32 accumulator updates deferred to the next block
                    if i == 1:
                        stt(z[3][0:64, :], pk[0:64, :], C[(3, 1)], ydup[0:64, :])
                        pending_accs = [
                            lambda pk=pk: stt(p45[:], pk[:], cv[:, 0:1], ydup[:]),
                            lambda pk=pk: stt(zb6t[64:128, :], pk[64:128, :], C[(6, 1)], ydup[64:128, :]),
                            lambda pk=pk: stt(ynewd[:], pk[:], HB[1], ydup[:]),
                        ]
                    elif i == 2:
                        stt(z[4][64:128, :], pk[64:128, :], C[(4, 2)], p45[64:128, :])
                        pending_accs = [
                            lambda pk=pk: stt(p45[0:64, :], pk[0:64, :], C[(5, 2)], p45[0:64, :]),
                            lambda pk=pk: stt(zb6t[64:128, :], pk[64:128, :], C[(6, 2)], zb6t[64:128, :]),
                            lambda pk=pk: stt(ynewd[:], pk[:], HB[2], ynewd[:]),
                        ]
                    elif i == 3:
                        stt(z[5][0:64, :], pk[0:64, :], C[(5, 3)], p45[0:64, :])
                        pending_accs = [
                            lambda pk=pk: stt(zb6t[64:128, :], pk[64:128, :], C[(6, 3)], zb6t[64:128, :]),
                            lambda pk=pk: stt(ynewd[:], pk[:], HB[3], ynewd[:]),
                        ]
                    elif i == 4:
                        stt(z[6][64:128, :], pk[64:128, :], C[(6, 4)], zb6t[64:128, :])
                        pending_accs = [
                            lambda pk=pk: stt(ynewd[:], pk[:], HB[4], ynewd[:]),
                        ]
                    elif i == 5:
                        # z1 for next step: y + sum_{j<=5} hb_j k_j (fp16),
                        # reads ynewd BEFORE its in-place hb5 update
                        if not last_step:
                            stt(z[1][0:64, :], pk[0:64, :], HB[5], ynewd[0:64, :])
                        pending_accs = [
                            lambda pk=pk: stt(ynewd[:], pk[:], HB[5], ynewd[:]),
                        ]
                    else:  # i == 6
                        if not last_step:
                            stt(z[2][0:64, :], pk[0:64, :], HB[6], ynewd[0:64, :])
                        stt(ydup[:], pk[:], HB[6], ynewd[:])

            nc.sync.dma_start(yout_d[:], ydup[0:64, :])

    nc.compile()
    return nc


def kernel(x0, u, W1, b1, W2, b2, W3, b3, t0, t1):
    from concourse.bass_utils import run_bass_kernel_spmd

    x0 = np.asarray(x0, dtype=np.float32)
    u = np.asarray(u, dtype=np.float32)
    W1 = np.asarray(W1, dtype=np.float32)
    W2 = np.asarray(W2, dtype=np.float32)
    W3 = np.asarray(W3, dtype=np.float32)
    b1 = np.asarray(b1, dtype=np.float32)
    b2 = np.asarray(b2, dtype=np.float32)
    b3 = np.asarray(b3, dtype=np.float32)

    Bt, D = x0.shape
    n = Bt // N_CORES
    h = DT0 * SECOND
    n_steps = int(round((float(np.asarray(t1)) - float(np.asarray(t0))) / h))
    b3_nonzero = bool(np.any(b3 != 0))

    nc = _build_program(n, n_steps, b3_nonzero)

    f16 = np.float16
    w1T = W1.T.astype(f16)  # [128, 256]
    w1t = np.ascontiguousarray(w1T)
    w1tf = np.ascontiguousarray(np.concatenate([w1T[64:128], w1T[0:64]], axis=0))
    w2T = W2.T.astype(f16)  # [256, 256]
    w2t = np.ascontiguousarray(
        np.concatenate([w2T[0:128, 0:128], w2T[0:128, 128:256], w2T[128:256, 0:128], w2T[128:256, 128:256]], axis=1)
    )
    w3T = W3.T.astype(f16)  # [256, 64]
    w3td = np.ascontiguousarray(
        np.concatenate([w3T[0:128], w3T[0:128], w3T[128:256], w3T[128:256]], axis=1)
    )

    # scaled FW = W1y@W3 variants for the fused layer3->layer1 ext matmuls;
    # variant j is emitted at stage j+1 (targets stage j+2, or stage 1 of the
    # next step for j=5)
    FW = (W1[:, 0:64] @ W3).astype(np.float32)  # [256, 256]
    cexts = [h * _A21, h * _A32, h * _A43, h * _A54, h * _A65, h * _B6]

    def lhst_cat(m):  # [256,256] -> [128,512] (k0m0|k0m1|k1m0|k1m1)
        mT = m.T.astype(np.float16)
        return np.ascontiguousarray(
            np.concatenate([mT[0:128, 0:128], mT[0:128, 128:256], mT[128:256, 0:128], mT[128:256, 128:256]], axis=1)
        )

    fws = [lhst_cat(c * FW) for c in cexts]

    c3 = W1[:, 0:64] @ b3  # [256]
    bb = np.zeros((128, 19), np.float32)
    bb[:, 0] = b1[0:128]
    bb[:, 1] = b1[128:256]
    for t in range(6):  # eff-b1 for stage t+1 (ext variant: t-1 mod 6)
        be = b1 + cexts[t - 1] * c3
        bb[:, 2 + 2 * t] = be[0:128]
        bb[:, 3 + 2 * t] = be[128:256]
    bb[:, 14] = b2[0:128]
    bb[:, 15] = b2[128:256]
    bb[0:64, 16] = b3
    bb[64:128, 16] = b3

    cvm = np.zeros((128, 1), np.float32)
    cvm[0:64, 0] = h * _A51
    cvm[64:128, 0] = h * _A41

    in_maps = []
    for c in range(N_CORES):
        sl = slice(c * n, (c + 1) * n)
        in_maps.append(
            {
                "y0": np.ascontiguousarray(x0[sl].T),
                "u16": np.ascontiguousarray(u[sl].T.astype(f16)),
                "w1t": w1t,
                "w1tf": w1tf,
                "w2t": w2t,
                "w3td": w3td,
                "bb": bb,
                "cv": cvm,
                **{f"fw{j}": fws[j] for j in range(6)},
            }
        )

    res = run_bass_kernel_spmd(nc, in_maps, list(range(N_CORES)))
    globals()["LAST_RESULT"] = res

    out = np.empty((Bt, D), np.float32)
    for c in range(N_CORES):
        out[c * n : (c + 1) * n, :] = res.results[c]["yout"].T
    return out



# revision 28
# speedup vs baseline: 1.3428x; 1.1626x over previous
"""Trainium2 Bass kernel for nn_NeuralODESolver (Tsit5 neural-ODE integrator).

Strategy (data-parallel across 8 NeuronCores, 2-way interleaved per core):
  - Shard the batch dim (1024) into 8 x 128; each core splits its batch into
    two fully independent 64-wide halves (A/B) whose Tsit5 chains interleave
    with a one-time startup skew: the tensor engine always has ready work, so
    the HAM clock gate holds the warm 2.4 GHz state. No tile is shared
    between halves (shared tiles re-couple the chains into lockstep).
  - FW-fused stage structure (4 chain hops): layer 3 is algebraically folded
    into the NEXT stage's layer 1 via FW = W1y@W3 (host-prescaled by the
    Butcher coefficient): pre1_t = W1cu@[zbase_t; I] + cext*FW@a2_{t-1}.
    The chain is relu1 -> L2 -> relu2 -> ext-matmuls; base matmuls, the L3
    dup-[k;k] and all Runge-Kutta scatters run off the critical path.
  - Bias/forcing folding: z tiles are [y; I64] (or flipped) in fp16; the L1
    stationary is [W1y^T ; (W1u@u + b1 + cext*W1y@b3)^T] so forcing and
    biases enter through the identity carrier -- u never ships, and both
    relus are single zero-bias merged [128, 2x64] ACT ops per half.
  - PSUM has_written is cleared bank-wide by any start=True matmul: each
    bank gets exactly one start=True (first MM); later regions write onto
    cleared elements with start=False, explicitly ordered after the clear.
  - RK scatters on DVE: one fp16 zbase write per stage (PSUM source) plus
    dup-paired fp32 accumulator updates (p45=[zb5;zb4], zb6, ynew, ydup).
"""

import numpy as np

# Tsitouras 5(4) tableau (5th-order weights; b7 = 0)
_A21 = 0.161
_A31, _A32 = -0.008480655492356989, 0.335480655492357
_A41, _A42, _A43 = 2.8971530571054935, -6.359448489975075, 4.3622954328695815
_A51, _A52, _A53, _A54 = 5.325864828439257, -11.748883564062828, 7.4955393428898365, -0.09249506636175525
_A61, _A62, _A63, _A64, _A65 = 5.86145544294642, -12.92096931784711, 8.159367898576159, -0.071584973281401, -0.028269050394068383
_B1, _B2, _B3, _B4, _B5, _B6 = 0.09646076681806523, 0.01, 0.4798896504144996, 1.379008574103742, -3.290069515436081, 2.324710524099774

SECOND = 1.0 / 3600.0
DT0 = 60.0

N_CORES = 8
NH = 64

N_WARMUP_MM = 18

_ZVAR = {1: 0, 2: 0, 3: 0, 4: 1, 5: 0, 6: 1}  # 0 = [y;I], 1 = [I;y]
_CEXT = [_A21, _A32, _A43, _A54, _A65, _B6]   # fw[i] coefficient (x h)


def _build_program(n_steps, b2_nonzero):
    import concourse.mybir as mybir
    import concourse.tile as tile
    from concourse.tile import add_dep_helper
    from concourse import bacc

    f16 = mybir.dt.float16
    f32 = mybir.dt.float32
    Relu = mybir.ActivationFunctionType.Relu
    MUL = mybir.AluOpType.mult
    ADD = mybir.AluOpType.add

    h = DT0 * SECOND
    C = {
        (3, 1): h * _A31,
        (4, 1): h * _A41, (4, 2): h * _A42,
        (5, 1): h * _A51, (5, 2): h * _A52, (5, 3): h * _A53,
        (6, 1): h * _A61, (6, 2): h * _A62, (6, 3): h * _A63, (6, 4): h * _A64,
    }
    HB = {j: h * v for j, v in enumerate((_B1, _B2, _B3, _B4, _B5, _B6), start=1)}

    nc = bacc.Bacc()

    y0_d = nc.declare_dram_parameter("y0", [64, 2 * NH], f32, isOutput=False)
    y016_d = nc.declare_dram_parameter("y016", [64, 2 * NH], f16, isOutput=False)
    id64_d = nc.declare_dram_parameter("id64", [64, 64], f16, isOutput=False)
    # L1 stationaries with identity-carrier biases: per stage j (carrier
    # includes the stage's eff-b1), per m-block, per half; plus a plain
    # variant for the step-0 prologue.
    keys = [(j, m, x) for j in list(range(1, 7)) + ["p"] for m in range(2) for x in range(2)]
    w1cu_d = {k: nc.declare_dram_parameter(f"w1cu{k[0]}{k[1]}{k[2]}", [128, 128], f16, isOutput=False)
              for k in keys}
    w2t_d = nc.declare_dram_parameter("w2t", [128, 512], f16, isOutput=False)
    w3td_d = nc.declare_dram_parameter("w3td", [128, 256], f16, isOutput=False)
    fw_d = [nc.declare_dram_parameter(f"fw{j}", [128, 512], f16, isOutput=False) for j in range(6)]
    cv_d = nc.declare_dram_parameter("cv", [128, 1], f32, isOutput=False)
    if b2_nonzero:
        cb2_d = nc.declare_dram_parameter("cb2", [128, 128], f16, isOutput=False)
    yout_d = nc.declare_dram_parameter("yout", [64, 2 * NH], f32, isOutput=True)

    with tile.TileContext(nc) as tc:
        with (
            tc.tile_pool(name="const", bufs=1) as cpool,
            tc.tile_pool(name="state", bufs=1) as spool,
            tc.tile_pool(name="act", bufs=2) as apool,
            tc.tile_pool(name="psum", bufs=2, space="PSUM") as ppool,
        ):
            w1cu = {k: cpool.tile([128, 128], f16, name=f"w1cu{k[0]}{k[1]}{k[2]}") for k in keys}
            w2t = cpool.tile([128, 512], f16)
            w3td = cpool.tile([128, 256], f16)
            fw = [cpool.tile([128, 512], f16, name=f"fw{j}") for j in range(6)]
            cv = cpool.tile([128, 1], f32)
            zerot = cpool.tile([128, 128], f16)
            if b2_nonzero:
                ident = cpool.tile([128, 128], f16)
                cb2 = cpool.tile([128, 128], f16)
                nc.sync.dma_start(ident[0:64, 0:64], id64_d[:])
                nc.sync.dma_start(ident[64:128, 64:128], id64_d[:])
                nc.gpsimd.memset(ident[0:64, 64:128], 0.0)
                nc.gpsimd.memset(ident[64:128, 0:64], 0.0)
                nc.sync.dma_start(cb2[:], cb2_d[:])

            nc.gpsimd.memset(zerot[:], 0.0)
            for i in range(N_WARMUP_MM):
                pwarm = ppool.tile([128, 128], f32, tag=f"pa1_{i % 2}", bufs=2, name="pwarm")
                nc.tensor.matmul(pwarm[:], zerot[:], zerot[:], start=True, stop=True)

            for k in keys:
                nc.sync.dma_start(w1cu[k][:], w1cu_d[k][:])
            nc.sync.dma_start(w2t[:], w2t_d[:])
            nc.sync.dma_start(w3td[:], w3td_d[:])
            for j in range(6):
                nc.sync.dma_start(fw[j][:], fw_d[j][:])
            nc.sync.dma_start(cv[:], cv_d[:])

            # ---- per-half state (NOTHING shared between halves) ----
            state = []
            for x in range(2):
                st = {}
                st["ydup"] = spool.tile([128, NH], f32, name=f"ydup{x}")
                st["ynew"] = spool.tile([128, NH], f32, name=f"ynew{x}")
                st["p45"] = spool.tile([128, NH], f32, name=f"p45_{x}")   # [zb5(0:64); zb4(64:128)]
                st["zb6"] = spool.tile([128, NH], f32, name=f"zb6_{x}")   # zb6 in 64:128
                st["z"] = {j: spool.tile([128, NH], f16, name=f"z{j}_{x}") for j in range(1, 7)}
                cols = slice(x * NH, (x + 1) * NH)
                nc.sync.dma_start(st["ydup"][0:64, :], y0_d[:, cols])
                nc.sync.dma_start(st["ydup"][64:128, :], y0_d[:, cols])
                nc.sync.dma_start(st["z"][1][0:64, :], y016_d[:, cols])
                nc.sync.dma_start(st["z"][2][0:64, :], y016_d[:, cols])
                for j in range(1, 7):
                    if _ZVAR[j] == 0:
                        nc.sync.dma_start(st["z"][j][64:128, :], id64_d[:])
                    else:
                        nc.sync.dma_start(st["z"][j][0:64, :], id64_d[:])
                state.append(st)

            def stt(out, in0, scal, in1):
                nc.vector.scalar_tensor_tensor(out, in0, scal, in1, op0=MUL, op1=ADD)

            LO = slice(0, 64)
            HI = slice(64, 128)
            skew = {"a_relu2": None, "done": False}

            def build_pa1(x, jt, a2, fwj):
                """Build pre1 for stage jt of half x: base + ext matmuls."""
                st = state[x]
                zt = st["z"][jt]
                key = (jt, 0, x)
                npa1 = ppool.tile([128, 2 * NH], f32, tag=f"pa1_{x}", bufs=2, name=f"pa1_{x}")
                mmb0 = nc.tensor.matmul(npa1[:, 0:NH], w1cu[key][:], zt[:], start=True, stop=False)
                mmb1 = nc.tensor.matmul(npa1[:, NH:2 * NH], w1cu[(jt, 1, x)][:], zt[:], start=False, stop=False,
                                        skip_group_check=True)
                add_dep_helper(mmb1.ins, mmb0.ins, sync=False, reason="bank clear order")
                nc.tensor.matmul(npa1[:, 0:NH], fwj[:, 0:128], a2[:, 0:NH], start=False, stop=False,
                                 skip_group_check=True)
                nc.tensor.matmul(npa1[:, NH:2 * NH], fwj[:, 128:256], a2[:, 0:NH], start=False, stop=False,
                                 skip_group_check=True)
                nc.tensor.matmul(npa1[:, 0:NH], fwj[:, 256:384], a2[:, NH:2 * NH], start=False, stop=True,
                                 skip_group_check=True)
                nc.tensor.matmul(npa1[:, NH:2 * NH], fwj[:, 384:512], a2[:, NH:2 * NH], start=False, stop=True,
                                 skip_group_check=True)
                return mmb0, npa1

            def emit_stage(x, j, step, last_step):
                st = state[x]
                ydup, ynew, p45, zb6 = st["ydup"], st["ynew"], st["p45"], st["zb6"]
                pa1 = st["pa1"]

                a1 = apool.tile([128, 2 * NH], f16, tag=f"a1_{x}", name=f"a1_{x}")
                nc.scalar.activation(a1[:], pa1[:], Relu)

                # L2 into the merged pa2 bank
                pa2 = ppool.tile([128, 2 * NH], f32, tag=f"pa2_{x}", bufs=1, name=f"pa2_{x}")
                if b2_nonzero:
                    mm_c = nc.tensor.matmul(pa2[:], ident[:], cb2[:], start=True, stop=False)
                    st2 = False
                else:
                    st2 = True
                mm_k0m0 = nc.tensor.matmul(pa2[:, 0:NH], w2t[:, 0:128], a1[:, 0:NH], start=st2, stop=False,
                                           skip_group_check=True)
                if x == 1 and not skew["done"] and skew["a_relu2"] is not None:
                    # one-time startup skew: hold half B ~half a stage behind
                    # half A so the chains dovetail instead of locking in phase
                    add_dep_helper(mm_k0m0.ins, skew["a_relu2"].ins, sync=True, reason="AB skew")
                    skew["done"] = True
                first = mm_c if b2_nonzero else mm_k0m0
                if b2_nonzero:
                    add_dep_helper(mm_k0m0.ins, mm_c.ins, sync=False, reason="bank clear order")
                mm_k0m1 = nc.tensor.matmul(pa2[:, NH:2 * NH], w2t[:, 128:256], a1[:, 0:NH], start=False, stop=False,
                                           skip_group_check=True)
                add_dep_helper(mm_k0m1.ins, first.ins, sync=False, reason="bank clear order")
                nc.tensor.matmul(pa2[:, 0:NH], w2t[:, 256:384], a1[:, NH:2 * NH], start=False, stop=True,
                                 skip_group_check=True)
                nc.tensor.matmul(pa2[:, NH:2 * NH], w2t[:, 384:512], a1[:, NH:2 * NH], start=False, stop=True,
                                 skip_group_check=True)

                a2 = apool.tile([128, 2 * NH], f16, tag=f"a2_{x}", name=f"a2_{x}")
                r2 = nc.scalar.activation(a2[:], pa2[:], Relu)
                if x == 0 and j == 1 and step == 0:
                    skew["a_relu2"] = r2

                # build the NEXT stage's pre1 (on-chain: its close gates relu1)
                if not (last_step and j == 6):
                    jt = j + 1 if j < 6 else 1
                    mmb0, npa1 = build_pa1(x, jt, a2, fw[j - 1])
                    st["pa1"] = npa1

                # L3: pk = [k; k] (own bank per half)
                pk = ppool.tile([128, NH], f32, tag=f"pk_{x}", bufs=1, name=f"pk_{x}")
                nc.tensor.matmul(pk[:], w3td[:, 0:128], a2[:, 0:NH], start=True, stop=False)
                nc.tensor.matmul(pk[:], w3td[:, 128:256], a2[:, NH:2 * NH], start=False, stop=True)

                # RK scatters (DVE): one fp16 zbase write + fp32 accumulators
                if j == 1:
                    stt(st["z"][3][LO, :], pk[LO, :], C[(3, 1)], ydup[LO, :])
                    stt(p45[:], pk[:], cv[:, 0:1], ydup[:])
                    stt(zb6[HI, :], pk[HI, :], C[(6, 1)], ydup[HI, :])
                    stt(ynew[:], pk[:], HB[1], ydup[:])
                elif j == 2:
                    stt(st["z"][4][HI, :], pk[HI, :], C[(4, 2)], p45[HI, :])
                    stt(p45[LO, :], pk[LO, :], C[(5, 2)], p45[LO, :])
                    stt(zb6[HI, :], pk[HI, :], C[(6, 2)], zb6[HI, :])
                    stt(ynew[:], pk[:], HB[2], ynew[:])
                elif j == 3:
                    stt(st["z"][5][LO, :], pk[LO, :], C[(5, 3)], p45[LO, :])
                    stt(zb6[HI, :], pk[HI, :], C[(6, 3)], zb6[HI, :])
                    stt(ynew[:], pk[:], HB[3], ynew[:])
                elif j == 4:
                    stt(st["z"][6][HI, :], pk[HI, :], C[(6, 4)], zb6[HI, :])
                    stt(ynew[:], pk[:], HB[4], ynew[:])
                elif j == 5:
                    if not last_step:
                        stt(st["z"][1][LO, :], pk[LO, :], HB[5], ynew[LO, :])
                    stt(ynew[:], pk[:], HB[5], ynew[:])
                else:  # j == 6
                    if not last_step:
                        stt(st["z"][2][LO, :], pk[LO, :], HB[6], ynew[LO, :])
                    stt(ydup[:], pk[:], HB[6], ynew[:])

            # prologue: full plain layer-1 for step-0 stage-1, both halves
            for x in range(2):
                st = state[x]
                pa1 = ppool.tile([128, 2 * NH], f32, tag=f"pa1_{x}", bufs=2, name=f"pa1_{x}")
                mm0 = nc.tensor.matmul(pa1[:, 0:NH], w1cu[("p", 0, x)][:], st["z"][1][:], start=True, stop=True)
                if x == 1:
                    pass  # skew applied on half B's first relu-gated work below
                mm1 = nc.tensor.matmul(pa1[:, NH:2 * NH], w1cu[("p", 1, x)][:], st["z"][1][:], start=False, stop=True,
                                       skip_group_check=True)
                add_dep_helper(mm1.ins, mm0.ins, sync=False, reason="bank clear order")
                st["pa1"] = pa1
                st["prologue_mm"] = mm0

            for step in range(n_steps):
                last_step = step == n_steps - 1
                for j in range(1, 7):
                    emit_stage(0, j, step, last_step)
                    emit_stage(1, j, step, last_step)

            for x in range(2):
                cols = slice(x * NH, (x + 1) * NH)
                nc.sync.dma_start(yout_d[:, cols], state[x]["ydup"][0:64, :])

    nc.compile()
    return nc


def kernel(x0, u, W1, b1, W2, b2, W3, b3, t0, t1):
    from concourse.bass_utils import run_bass_kernel_spmd

    x0 = np.asarray(x0, dtype=np.float32)
    u = np.asarray(u, dtype=np.float32)
    W1 = np.asarray(W1, dtype=np.float32)
    W2 = np.asarray(W2, dtype=np.float32)
    W3 = np.asarray(W3, dtype=np.float32)
    b1 = np.asarray(b1, dtype=np.float32)
    b2 = np.asarray(b2, dtype=np.float32)
    b3 = np.asarray(b3, dtype=np.float32)

    Bt, D = x0.shape
    n = Bt // N_CORES
    h = DT0 * SECOND
    n_steps = int(round((float(np.asarray(t1)) - float(np.asarray(t0))) / h))
    b2_nonzero = bool(np.any(b2 != 0))

    nc = _build_program(n_steps, b2_nonzero)

    f16 = np.float16
    W1y = W1[:, 0:64]
    W1u = W1[:, 64:128]

    w2T = W2.T.astype(f16)
    w2t = np.ascontiguousarray(
        np.concatenate([w2T[0:128, 0:128], w2T[0:128, 128:256], w2T[128:256, 0:128], w2T[128:256, 128:256]], axis=1)
    )
    w3T = W3.T.astype(f16)
    w3td = np.ascontiguousarray(
        np.concatenate([w3T[0:128], w3T[0:128], w3T[128:256], w3T[128:256]], axis=1)
    )

    FW = (W1y @ W3).astype(np.float32)  # [256, 256]
    cexts = [h * c for c in _CEXT]

    def lhst_cat(m):  # [256,256] -> [128,512] (k0m0|k0m1|k1m0|k1m1)
        mT = m.T.astype(np.float16)
        return np.ascontiguousarray(
            np.concatenate([mT[0:128, 0:128], mT[0:128, 128:256], mT[128:256, 0:128], mT[128:256, 128:256]], axis=1)
        )

    fws = [lhst_cat(c * FW) for c in cexts]

    c3 = W1y @ b3  # [256] eff-b1 correction per stage

    cvm = np.zeros((128, 1), np.float32)
    cvm[0:64, 0] = h * _A51
    cvm[64:128, 0] = h * _A41

    id64 = np.eye(64, dtype=f16)

    in_maps = []
    for c in range(N_CORES):
        sl = slice(c * n, (c + 1) * n)
        x0c = x0[sl]
        uc = u[sl]
        im = {
            "y0": np.ascontiguousarray(x0c.T),
            "y016": np.ascontiguousarray(x0c.T.astype(f16)),
            "id64": id64,
            "w2t": w2t,
            "w3td": w3td,
            "cv": cvm,
            **{f"fw{j}": fws[j] for j in range(6)},
        }
        if b2_nonzero:
            cb2 = np.zeros((128, 128), np.float32)
            cb2[:, 0:64] = b2[0:128, None]
            cb2[:, 64:128] = b2[128:256, None]
            im["cb2"] = cb2.astype(f16)
        for x in range(2):
            ux = uc[x * NH:(x + 1) * NH]
            cu1 = W1u @ ux.T + b1[:, None]  # [256, 64]
            for jkey in list(range(1, 7)) + ["p"]:
                if jkey == "p":
                    cu1e = cu1
                    v = _ZVAR[1]
                else:
                    cu1e = cu1 + cexts[(jkey - 2) % 6] * c3[:, None]
                    v = _ZVAR[jkey]
                for m in range(2):
                    w1yT = W1y.T[:, m * 128:(m + 1) * 128]
                    cu1T = cu1e[m * 128:(m + 1) * 128, :].T
                    if v == 0:
                        blk = np.concatenate([w1yT, cu1T], axis=0)
                    else:
                        blk = np.concatenate([cu1T, w1yT], axis=0)
                    im[f"w1cu{jkey}{m}{x}"] = np.ascontiguousarray(blk.astype(f16))
        in_maps.append(im)

    res = run_bass_kernel_spmd(nc, in_maps, list(range(N_CORES)))
    globals()["LAST_RESULT"] = res

    out = np.empty((Bt, D), np.float32)
    for c in range(N_CORES):
        out[c * n:(c + 1) * n, :] = res.results[c]["yout"].T
    return out
